# revision 1
# baseline (speedup 1.0000x reference)
"""KVGather kernel for Trainium2 (8 NeuronCores, SPMD data-parallel over batch).

Problem: kv (16, 64, 196, 128) f32; r_idx/r_weight (16, 64, 4).
out[n, p, t] = r_weight[n, p, t] * kv[n, r_idx[n, p, t]]  -> (16, 64, 4, 196, 128)

Strategy (per core: 2 batches):
  - Gather is done on the PE array as a one-hot matmul: psum[m, :] =
    sel_mh.T @ rhs_chunk, where sel is a host-built {0,1} selection matrix
    and rhs holds the batch's kv regions flat-packed across 128 partitions
    (partition h*64 + r = half h of region r).
  - kv is host-decomposed into three bf16 terms (hi/mid/lo), which is a
    bit-exact representation of fp32 for this data; the three bf16 matmuls
    accumulate in fp32 PSUM, reconstructing the gathered fp32 exactly while
    running the PE at 1 cycle/row (vs 4 for fp32 matmuls).
  - PSUM eviction fuses the r_weight multiply (tensor_scalar with a
    per-partition scalar = per-gather weight), alternating DVE/ACT.
  - Output DMAs are [128 x chunk] f32 with 2 KB contiguous runs per
    partition (chunk cols are contiguous within each gather's flat region).

Everything is static: one compiled program for all cores and all inputs;
indices/weights only enter through input tensors (sel, wt).
"""

import sys

if "/opt/trn_rl_repo" not in sys.path:
    sys.path.insert(0, "/opt/trn_rl_repo")

import numpy as np
import ml_dtypes

import concourse.bass as bass
import concourse.bacc as bacc
import concourse.mybir as mybir
from concourse import tile
from concourse.bass_utils import run_bass_kernel_spmd

BF16 = ml_dtypes.bfloat16

# Problem constants
N, P2, TOPK, W2, C_KV = 16, 64, 4, 196, 128
REG = W2 * C_KV  # 25088 f32 per region
RHALF = REG // 2  # 12544 per region half
N_CORES = 8
B = N // N_CORES  # batches per core = 2
G = P2 * TOPK  # gathers per batch = 256
MG = G // 128  # m-groups of 128 gathers = 2
CH = 512  # psum chunk (one bank of f32)
NCH = (RHALF + CH - 1) // CH  # 25 chunks (24x512 + 256)

_COMPILED = None
RUN_KWARGS = {}  # test harness may set e.g. {"trace": True}
LAST_RESULTS = None  # BassKernelResults of the last run (for profiling)


def _build():
    nc = bacc.Bacc("TRN2", target_bir_lowering=False, debug=False, num_devices=N_CORES)
    f32, bf16 = mybir.dt.float32, mybir.dt.bfloat16

    hi_d = nc.dram_tensor("hi", [B, 128, RHALF], bf16, kind="ExternalInput").ap()
    mid_d = nc.dram_tensor("mid", [B, 128, RHALF], bf16, kind="ExternalInput").ap()
    lo_d = nc.dram_tensor("lo", [B, 128, RHALF], bf16, kind="ExternalInput").ap()
    sel_d = nc.dram_tensor("sel", [128, B * MG * 2 * 128], bf16, kind="ExternalInput").ap()
    wt_d = nc.dram_tensor("wt", [128, B * MG], f32, kind="ExternalInput").ap()
    out_d = nc.dram_tensor("out", [B, G, REG], f32, kind="ExternalOutput").ap()

    terms_d = [hi_d, mid_d, lo_d]

    with tile.TileContext(nc) as tc:
        with (
            tc.tile_pool(name="rhs", bufs=2) as rhs_pool,
            tc.tile_pool(name="const", bufs=1) as const_pool,
            tc.tile_pool(name="psum", bufs=8, space="PSUM") as psum_pool,
            tc.tile_pool(name="outp", bufs=6) as out_pool,
        ):
            sel_sb = const_pool.tile([128, B * MG * 2 * 128], bf16)
            wt_sb = const_pool.tile([128, B * MG], f32)
            nc.sync.dma_start(sel_sb[:], sel_d)
            nc.sync.dma_start(wt_sb[:], wt_d)

            # chunk-aligned column stripes so the first matmuls only wait on
            # the first stripe of each term, not the whole 3.2 MB load
            stripes = [(0, 3584), (3584, 6656), (6656, 9728), (9728, RHALF)]
            for b in range(B):
                hi_sb = rhs_pool.tile([128, RHALF], bf16, tag="term0")
                mid_sb = rhs_pool.tile([128, RHALF], bf16, tag="term1")
                lo_sb = rhs_pool.tile([128, RHALF], bf16, tag="term2")
                term_sb = [hi_sb, mid_sb, lo_sb]
                for s0, s1 in stripes:
                    for ti, td in enumerate(terms_d):
                        nc.sync.dma_start(term_sb[ti][:, s0:s1], td[b][:, s0:s1])

                ev = 0
                for mg in range(MG):
                    wcol = wt_sb[:, b * MG + mg : b * MG + mg + 1]
                    for h in range(2):
                        si = (b * MG + mg) * 2 + h
                        sel_ap = sel_sb[:, si * 128 : (si + 1) * 128]
                        for c in range(NCH):
                            cw = min(CH, RHALF - c * CH)
                            ps = psum_pool.tile([128, cw], f32, tag="ps")
                            for ti in range(3):
                                nc.tensor.matmul(
                                    ps[:],
                                    sel_ap,
                                    term_sb[ti][:, c * CH : c * CH + cw],
                                    start=(ti == 0),
                                    stop=(ti == 2),
                                )
                            ot = out_pool.tile([128, cw], f32, tag="ot")
                            if ev % 2 == 0:
                                nc.vector.tensor_scalar_mul(ot[:], ps[:], wcol)
                            else:
                                nc.scalar.activation(
                                    ot[:],
                                    ps[:],
                                    mybir.ActivationFunctionType.Copy,
                                    scale=wcol,
                                )
                            ev += 1
                            dst = out_d[
                                b,
                                mg * 128 : (mg + 1) * 128,
                                h * RHALF + c * CH : h * RHALF + c * CH + cw,
                            ]
                            nc.sync.dma_start(dst, ot[:])

    nc.compile()
    return nc


def _get_nc():
    global _COMPILED
    if _COMPILED is None:
        _COMPILED = _build()
    return _COMPILED


def _prep_core(kv_c: np.ndarray, idx_c: np.ndarray, w_c: np.ndarray) -> dict:
    """kv_c (B, 64, 196, 128) f32, idx_c (B, 64, 4) int, w_c (B, 64, 4) f32."""
    # rhs layout [B, 128, RHALF]: partition h*64 + r = half h of region r (flat)
    kvr = (
        kv_c.reshape(B, P2, 2, RHALF).transpose(0, 2, 1, 3).reshape(B, 128, RHALF)
    ).astype(np.float32)
    hi = kvr.astype(BF16)
    r1 = kvr - hi.astype(np.float32)
    mid = r1.astype(BF16)
    lo = (r1 - mid.astype(np.float32)).astype(BF16)

    idx_f = idx_c.reshape(B, G).astype(np.int64)
    w_f = w_c.reshape(B, G).astype(np.float32)

    sel = np.zeros((128, B, MG, 2, 128), dtype=BF16)
    k = np.arange(128)[:, None]
    for b in range(B):
        for mg in range(MG):
            im = idx_f[b, mg * 128 : (mg + 1) * 128][None, :]
            sel[:, b, mg, 0] = (k == im).astype(BF16)
            sel[:, b, mg, 1] = (k == im + 64).astype(BF16)
    sel = sel.reshape(128, B * MG * 2 * 128)

    wt = np.zeros((128, B * MG), dtype=np.float32)
    for b in range(B):
        for mg in range(MG):
            wt[:, b * MG + mg] = w_f[b, mg * 128 : (mg + 1) * 128]

    return {"hi": hi, "mid": mid, "lo": lo, "sel": sel, "wt": wt}


def kernel(r_idx: np.ndarray, r_weight: np.ndarray, kv: np.ndarray) -> np.ndarray:
    global LAST_RESULTS
    nc = _get_nc()
    kv = np.asarray(kv, dtype=np.float32)
    r_idx = np.asarray(r_idx)
    r_weight = np.asarray(r_weight, dtype=np.float32)

    in_maps = [
        _prep_core(
            kv[c * B : (c + 1) * B],
            r_idx[c * B : (c + 1) * B],
            r_weight[c * B : (c + 1) * B],
        )
        for c in range(N_CORES)
    ]

    res = run_bass_kernel_spmd(nc, in_maps, core_ids=list(range(N_CORES)), **RUN_KWARGS)
    LAST_RESULTS = res

    out = np.empty((N, P2, TOPK, W2, C_KV), dtype=np.float32)
    for c in range(N_CORES):
        o = res.results[c]["out"]  # (B, G, REG)
        out[c * B : (c + 1) * B] = o.reshape(B, P2, TOPK, W2, C_KV)
    return out



# revision 2
# speedup vs baseline: 1.5850x; 1.5850x over previous
"""KVGather kernel for Trainium2 (8 NeuronCores, SPMD data-parallel over batch).

Problem: kv (16, 64, 196, 128) f32; r_idx/r_weight (16, 64, 4).
out[n, p, t] = r_weight[n, p, t] * kv[n, r_idx[n, p, t]]  -> (16, 64, 4, 196, 128)

Strategy (per core: 2 batches). The kernel is HBM-bandwidth bound, so both
sides of the traffic run in bf16 (tolerance is 2e-2; bf16 end-to-end keeps
max rel err ~6e-3):
  - kv is loaded once per batch as a single bf16 tensor [128, 12544]
    (partition h*64 + r = half h of region r, flat over (w2, c_kv)).
  - Gather runs on the PE array as a one-hot matmul: psum[m, :] =
    sel_mh.T @ kv_chunk, with sel a host-built {0,1} bf16 selection matrix.
  - PSUM eviction fuses the r_weight multiply (per-partition f32 scalar)
    and the f32->bf16 downconvert, alternating DVE/ACT 5:3 to match their
    throughput ratio.
  - Output is written bf16: one 6.4 MB DMA per (batch, group of 128
    gathers), 50 KB contiguous per partition; host upconverts to f32.

Per-core HBM traffic: 6.4 MB in + 25.7 MB out = 32.1 MB (vs 51.4+19.3 for
the f32-out/3-term baseline), against a ~358 GB/s per-core HBM limit.

Everything is static: one compiled program for all cores and all inputs;
indices/weights only enter through input tensors (sel, wt).
"""

import sys

if "/opt/trn_rl_repo" not in sys.path:
    sys.path.insert(0, "/opt/trn_rl_repo")

import numpy as np
import ml_dtypes

import concourse.bass as bass
import concourse.bacc as bacc
import concourse.mybir as mybir
from concourse import tile
from concourse.bass_utils import run_bass_kernel_spmd

BF16 = ml_dtypes.bfloat16

# Problem constants
N, P2, TOPK, W2, C_KV = 16, 64, 4, 196, 128
REG = W2 * C_KV  # 25088 elems per region
RHALF = REG // 2  # 12544 per region half
N_CORES = 8
B = N // N_CORES  # batches per core = 2
G = P2 * TOPK  # gathers per batch = 256
MG = G // 128  # m-groups of 128 gathers = 2
CH = 1792  # psum chunk: 3.5 banks of f32, 7 equal chunks per half
NCH = RHALF // CH  # 7
MM = 512  # max moving free dim per matmul

_COMPILED = None
RUN_KWARGS = {}  # test harness may set e.g. {"trace": True}
LAST_RESULTS = None  # BassKernelResults of the last run (for profiling)


def _build():
    nc = bacc.Bacc("TRN2", target_bir_lowering=False, debug=False, num_devices=N_CORES)
    f32, bf16 = mybir.dt.float32, mybir.dt.bfloat16

    kv_d = nc.dram_tensor("kvb", [B, 128, RHALF], bf16, kind="ExternalInput").ap()
    sel_d = nc.dram_tensor("sel", [128, B * MG * 2 * 128], bf16, kind="ExternalInput").ap()
    wt_d = nc.dram_tensor("wt", [128, B * MG], f32, kind="ExternalInput").ap()
    out_d = nc.dram_tensor("out", [B, G, REG], bf16, kind="ExternalOutput").ap()

    with tile.TileContext(nc) as tc:
        with (
            tc.tile_pool(name="rhs", bufs=2) as rhs_pool,
            tc.tile_pool(name="const", bufs=1) as const_pool,
            tc.tile_pool(name="psum", bufs=2, space="PSUM") as psum_pool,
            tc.tile_pool(name="outp", bufs=2) as out_pool,
        ):
            sel_sb = const_pool.tile([128, B * MG * 2 * 128], bf16)
            wt_sb = const_pool.tile([128, B * MG], f32)
            nc.sync.dma_start(sel_sb[:], sel_d)
            nc.sync.dma_start(wt_sb[:], wt_d)

            # chunk-aligned column stripes (multiples of CH) so the first
            # matmuls only wait on the first stripe, not the whole 3.2 MB
            stripes = [(0, 3584), (3584, 7168), (7168, 10752), (10752, RHALF)]
            ev = 0
            for b in range(B):
                kv_sb = rhs_pool.tile([128, RHALF], bf16, tag="term")
                for s0, s1 in stripes:
                    nc.sync.dma_start(kv_sb[:, s0:s1], kv_d[b][:, s0:s1])

                for mg in range(MG):
                    wcol = wt_sb[:, b * MG + mg : b * MG + mg + 1]
                    ot = out_pool.tile([128, REG], bf16, tag="ot")
                    for h in range(2):
                        si = (b * MG + mg) * 2 + h
                        sel_ap = sel_sb[:, si * 128 : (si + 1) * 128]
                        for c in range(NCH):
                            ps = psum_pool.tile([128, CH], f32, tag="ps")
                            for m0 in range(0, CH, MM):
                                mw = min(MM, CH - m0)
                                col = c * CH + m0
                                nc.tensor.matmul(
                                    ps[:, m0 : m0 + mw],
                                    sel_ap,
                                    kv_sb[:, col : col + mw],
                                )
                            dst = ot[:, h * RHALF + c * CH : h * RHALF + (c + 1) * CH]
                            # alternate DVE/ACT 5:3 (~245 vs ~153 G elem/s)
                            if ev % 8 in (0, 2, 3, 5, 6):
                                nc.vector.tensor_scalar_mul(dst, ps[:], wcol)
                            else:
                                nc.scalar.activation(
                                    dst,
                                    ps[:],
                                    mybir.ActivationFunctionType.Copy,
                                    scale=wcol,
                                )
                            ev += 1
                    nc.sync.dma_start(out_d[b, mg * 128 : (mg + 1) * 128, :], ot[:])

    nc.compile()
    return nc


def _get_nc():
    global _COMPILED
    if _COMPILED is None:
        _COMPILED = _build()
    return _COMPILED


def _prep_core(kv_c: np.ndarray, idx_c: np.ndarray, w_c: np.ndarray) -> dict:
    """kv_c (B, 64, 196, 128) f32, idx_c (B, 64, 4) int, w_c (B, 64, 4) f32."""
    # rhs layout [B, 128, RHALF]: partition h*64 + r = half h of region r (flat)
    kvb = (
        kv_c.reshape(B, P2, 2, RHALF).transpose(0, 2, 1, 3).reshape(B, 128, RHALF)
    ).astype(BF16)

    idx_f = idx_c.reshape(B, G).astype(np.int64)
    w_f = w_c.reshape(B, G).astype(np.float32)

    sel = np.zeros((128, B, MG, 2, 128), dtype=BF16)
    k = np.arange(128)[:, None]
    for b in range(B):
        for mg in range(MG):
            im = idx_f[b, mg * 128 : (mg + 1) * 128][None, :]
            sel[:, b, mg, 0] = (k == im).astype(BF16)
            sel[:, b, mg, 1] = (k == im + 64).astype(BF16)
    sel = sel.reshape(128, B * MG * 2 * 128)

    wt = np.zeros((128, B * MG), dtype=np.float32)
    for b in range(B):
        for mg in range(MG):
            wt[:, b * MG + mg] = w_f[b, mg * 128 : (mg + 1) * 128]

    return {"kvb": kvb, "sel": sel, "wt": wt}


def kernel(r_idx: np.ndarray, r_weight: np.ndarray, kv: np.ndarray) -> np.ndarray:
    global LAST_RESULTS
    nc = _get_nc()
    kv = np.asarray(kv, dtype=np.float32)
    r_idx = np.asarray(r_idx)
    r_weight = np.asarray(r_weight, dtype=np.float32)

    in_maps = [
        _prep_core(
            kv[c * B : (c + 1) * B],
            r_idx[c * B : (c + 1) * B],
            r_weight[c * B : (c + 1) * B],
        )
        for c in range(N_CORES)
    ]

    res = run_bass_kernel_spmd(nc, in_maps, core_ids=list(range(N_CORES)), **RUN_KWARGS)
    LAST_RESULTS = res

    out = np.empty((N, P2, TOPK, W2, C_KV), dtype=np.float32)
    for c in range(N_CORES):
        o = res.results[c]["out"]  # (B, G, REG) bf16
        out[c * B : (c + 1) * B] = o.astype(np.float32).reshape(B, P2, TOPK, W2, C_KV)
    return out


# revision 3
# speedup vs baseline: 1.8356x; 1.1582x over previous
"""KVGather kernel for Trainium2 (8 NeuronCores, SPMD data-parallel over batch).

Problem: kv (16, 64, 196, 128) f32; r_idx/r_weight (16, 64, 4).
out[n, p, t] = r_weight[n, p, t] * kv[n, r_idx[n, p, t]]  -> (16, 64, 4, 196, 128)

Strategy (per core: 2 batches). The kernel is HBM-bandwidth bound, so both
sides of the traffic run in bf16 (tolerance is 2e-2; bf16 end-to-end keeps
max rel err ~6e-3):
  - kv is loaded once per batch as a single bf16 tensor [128, 12544]
    (partition h*64 + r = half h of region r, flat over (w2, c_kv)).
  - Gather runs on the PE array as a one-hot matmul: psum[m, :] =
    sel_mh.T @ kv_chunk, with sel a host-built {0,1} bf16 selection matrix.
  - PSUM eviction fuses the r_weight multiply (per-partition f32 scalar)
    and the f32->bf16 downconvert, alternating DVE/ACT 5:3 to match their
    throughput ratio.
  - Output is written bf16: one 6.4 MB DMA per (batch, group of 128
    gathers), 50 KB contiguous per partition; host upconverts to f32.

Per-core HBM traffic: 6.4 MB in + 25.7 MB out = 32.1 MB (vs 51.4+19.3 for
the f32-out/3-term baseline), against a ~358 GB/s per-core HBM limit.

Everything is static: one compiled program for all cores and all inputs;
indices/weights only enter through input tensors (sel, wt).
"""

import sys

if "/opt/trn_rl_repo" not in sys.path:
    sys.path.insert(0, "/opt/trn_rl_repo")

import numpy as np
import ml_dtypes

import concourse.bass as bass
import concourse.bacc as bacc
import concourse.mybir as mybir
from concourse import tile
from concourse.bass_utils import run_bass_kernel_spmd

BF16 = ml_dtypes.bfloat16

# Problem constants
N, P2, TOPK, W2, C_KV = 16, 64, 4, 196, 128
REG = W2 * C_KV  # 25088 elems per region
RHALF = REG // 2  # 12544 per region half
N_CORES = 8
B = N // N_CORES  # batches per core = 2
G = P2 * TOPK  # gathers per batch = 256
MG = G // 128  # m-groups of 128 gathers = 2
CH = 1792  # psum chunk: 3.5 banks of f32, 7 equal chunks per half
NCH = RHALF // CH  # 7
MM = 512  # max moving free dim per matmul

_COMPILED = None
RUN_KWARGS = {}  # test harness may set e.g. {"trace": True}
LAST_RESULTS = None  # BassKernelResults of the last run (for profiling)


def _build():
    nc = bacc.Bacc("TRN2", target_bir_lowering=False, debug=False, num_devices=N_CORES)
    f32, bf16 = mybir.dt.float32, mybir.dt.bfloat16

    kv_d = nc.dram_tensor("kvb", [B, 128, RHALF], bf16, kind="ExternalInput").ap()
    sel_d = nc.dram_tensor("sel", [128, B * MG * 2 * 128], bf16, kind="ExternalInput").ap()
    wt_d = nc.dram_tensor("wt", [128, B * MG], f32, kind="ExternalInput").ap()
    out_d = nc.dram_tensor("out", [B, G, REG], bf16, kind="ExternalOutput").ap()

    with tile.TileContext(nc) as tc:
        with (
            tc.tile_pool(name="rhs", bufs=2) as rhs_pool,
            tc.tile_pool(name="const", bufs=1) as const_pool,
            tc.tile_pool(name="psum", bufs=2, space="PSUM") as psum_pool,
            tc.tile_pool(name="outp", bufs=2) as out_pool,
        ):
            sel_sb = const_pool.tile([128, B * MG * 2 * 128], bf16)
            wt_sb = const_pool.tile([128, B * MG], f32)
            nc.sync.dma_start(sel_sb[:], sel_d)
            nc.sync.dma_start(wt_sb[:], wt_d)

            # Load all kv upfront on the SWDGE path (gpsimd) so input DMAs
            # never queue behind output DMAs on the HWDGE FIFO ring; stripes
            # are chunk-aligned so the first matmuls only wait on stripe 0.
            stripes = [(0, 3584), (3584, 7168), (7168, 10752), (10752, RHALF)]
            kv_sbs = []
            for b in range(B):
                kv_sb = rhs_pool.tile([128, RHALF], bf16, tag="term", name=f"kv{b}")
                kv_sbs.append(kv_sb)
                for s0, s1 in stripes:
                    nc.gpsimd.dma_start(kv_sb[:, s0:s1], kv_d[b][:, s0:s1])

            ev = 0
            for b in range(B):
                kv_sb = kv_sbs[b]
                for mg in range(MG):
                    wcol = wt_sb[:, b * MG + mg : b * MG + mg + 1]
                    ot = out_pool.tile([128, REG], bf16, tag="ot")
                    for h in range(2):
                        si = (b * MG + mg) * 2 + h
                        sel_ap = sel_sb[:, si * 128 : (si + 1) * 128]
                        for c in range(NCH):
                            ps = psum_pool.tile([128, CH], f32, tag="ps")
                            for m0 in range(0, CH, MM):
                                mw = min(MM, CH - m0)
                                col = c * CH + m0
                                nc.tensor.matmul(
                                    ps[:, m0 : m0 + mw],
                                    sel_ap,
                                    kv_sb[:, col : col + mw],
                                )
                            dst = ot[:, h * RHALF + c * CH : h * RHALF + (c + 1) * CH]
                            # alternate DVE/ACT 1:1 (both ~110 G elem/s on
                            # f32 PSUM reads)
                            if ev % 2 == 0:
                                nc.vector.tensor_scalar_mul(dst, ps[:], wcol)
                            else:
                                nc.scalar.activation(
                                    dst,
                                    ps[:],
                                    mybir.ActivationFunctionType.Copy,
                                    scale=wcol,
                                )
                            ev += 1
                        nc.sync.dma_start(
                            out_d[
                                b,
                                mg * 128 : (mg + 1) * 128,
                                h * RHALF : (h + 1) * RHALF,
                            ],
                            ot[:, h * RHALF : (h + 1) * RHALF],
                        )

    nc.compile()
    return nc


def _get_nc():
    global _COMPILED
    if _COMPILED is None:
        _COMPILED = _build()
    return _COMPILED


def _prep_core(kv_c: np.ndarray, idx_c: np.ndarray, w_c: np.ndarray) -> dict:
    """kv_c (B, 64, 196, 128) f32, idx_c (B, 64, 4) int, w_c (B, 64, 4) f32."""
    # rhs layout [B, 128, RHALF]: partition h*64 + r = half h of region r (flat)
    kvb = (
        kv_c.reshape(B, P2, 2, RHALF).transpose(0, 2, 1, 3).reshape(B, 128, RHALF)
    ).astype(BF16)

    idx_f = idx_c.reshape(B, G).astype(np.int64)
    w_f = w_c.reshape(B, G).astype(np.float32)

    sel = np.zeros((128, B, MG, 2, 128), dtype=BF16)
    k = np.arange(128)[:, None]
    for b in range(B):
        for mg in range(MG):
            im = idx_f[b, mg * 128 : (mg + 1) * 128][None, :]
            sel[:, b, mg, 0] = (k == im).astype(BF16)
            sel[:, b, mg, 1] = (k == im + 64).astype(BF16)
    sel = sel.reshape(128, B * MG * 2 * 128)

    wt = np.zeros((128, B * MG), dtype=np.float32)
    for b in range(B):
        for mg in range(MG):
            wt[:, b * MG + mg] = w_f[b, mg * 128 : (mg + 1) * 128]

    return {"kvb": kvb, "sel": sel, "wt": wt}


def kernel(r_idx: np.ndarray, r_weight: np.ndarray, kv: np.ndarray) -> np.ndarray:
    global LAST_RESULTS
    nc = _get_nc()
    kv = np.asarray(kv, dtype=np.float32)
    r_idx = np.asarray(r_idx)
    r_weight = np.asarray(r_weight, dtype=np.float32)

    in_maps = [
        _prep_core(
            kv[c * B : (c + 1) * B],
            r_idx[c * B : (c + 1) * B],
            r_weight[c * B : (c + 1) * B],
        )
        for c in range(N_CORES)
    ]

    res = run_bass_kernel_spmd(nc, in_maps, core_ids=list(range(N_CORES)), **RUN_KWARGS)
    LAST_RESULTS = res

    out = np.empty((N, P2, TOPK, W2, C_KV), dtype=np.float32)
    for c in range(N_CORES):
        o = res.results[c]["out"]  # (B, G, REG) bf16
        out[c * B : (c + 1) * B] = o.astype(np.float32).reshape(B, P2, TOPK, W2, C_KV)
    return out


# revision 5
# speedup vs baseline: 1.8680x; 1.0176x over previous
"""KVGather kernel for Trainium2 (8 NeuronCores, SPMD data-parallel over batch).

Problem: kv (16, 64, 196, 128) f32; r_idx/r_weight (16, 64, 4).
out[n, p, t] = r_weight[n, p, t] * kv[n, r_idx[n, p, t]]  -> (16, 64, 4, 196, 128)

Strategy (per core: 2 batches). The kernel is HBM-bandwidth bound, so both
sides of the traffic run in bf16 (tolerance is 2e-2; bf16 end-to-end keeps
max rel err ~6e-3):
  - kv is loaded once per batch as a single bf16 tensor [128, 12544]
    (partition h*64 + r = half h of region r, flat over (w2, c_kv)).
  - Gather runs on the PE array as a one-hot matmul: psum[m, :] =
    sel_mh.T @ kv_chunk, with sel a host-built {0,1} bf16 selection matrix.
  - PSUM eviction fuses the r_weight multiply (per-partition f32 scalar)
    and the f32->bf16 downconvert, alternating DVE/ACT 5:3 to match their
    throughput ratio.
  - Output is written bf16: one 6.4 MB DMA per (batch, group of 128
    gathers), 50 KB contiguous per partition; host upconverts to f32.

Per-core HBM traffic: 6.4 MB in + 25.7 MB out = 32.1 MB (vs 51.4+19.3 for
the f32-out/3-term baseline), against a ~358 GB/s per-core HBM limit.

Everything is static: one compiled program for all cores and all inputs;
indices/weights only enter through input tensors (sel, wt).
"""

import sys

if "/opt/trn_rl_repo" not in sys.path:
    sys.path.insert(0, "/opt/trn_rl_repo")

import numpy as np
import ml_dtypes

import concourse.bass as bass
import concourse.bacc as bacc
import concourse.mybir as mybir
from concourse import tile
from concourse.bass_utils import run_bass_kernel_spmd

BF16 = ml_dtypes.bfloat16

# Problem constants
N, P2, TOPK, W2, C_KV = 16, 64, 4, 196, 128
REG = W2 * C_KV  # 25088 elems per region
RHALF = REG // 2  # 12544 per region half
N_CORES = 8
B = N // N_CORES  # batches per core = 2
G = P2 * TOPK  # gathers per batch = 256
MG = G // 128  # m-groups of 128 gathers = 2
CH = 1792  # psum chunk: 3.5 banks of f32, 7 equal chunks per half
NCH = RHALF // CH  # 7
MM = 512  # max moving free dim per matmul

_COMPILED = None
RUN_KWARGS = {}  # test harness may set e.g. {"trace": True}
LAST_RESULTS = None  # BassKernelResults of the last run (for profiling)


def _build():
    nc = bacc.Bacc("TRN2", target_bir_lowering=False, debug=False, num_devices=N_CORES)
    f32, bf16 = mybir.dt.float32, mybir.dt.bfloat16

    kv_d = nc.dram_tensor("kvb", [B, 128, RHALF], bf16, kind="ExternalInput").ap()
    sel_d = nc.dram_tensor("sel", [128, B * MG * 2 * 128], bf16, kind="ExternalInput").ap()
    wt_d = nc.dram_tensor("wt", [128, B * MG], f32, kind="ExternalInput").ap()
    out_d = nc.dram_tensor("out", [B, G, REG], bf16, kind="ExternalOutput").ap()

    with tile.TileContext(nc) as tc:
        with (
            tc.tile_pool(name="rhs", bufs=2) as rhs_pool,
            tc.tile_pool(name="const", bufs=1) as const_pool,
            tc.tile_pool(name="psum", bufs=2, space="PSUM") as psum_pool,
            tc.tile_pool(name="outp", bufs=2) as out_pool,
        ):
            sel_sb = const_pool.tile([128, B * MG * 2 * 128], bf16)
            wt_sb = const_pool.tile([128, B * MG], f32)
            nc.sync.dma_start(sel_sb[:], sel_d)
            nc.sync.dma_start(wt_sb[:], wt_d)

            # Load all kv upfront on the scalar HWDGE ring so input DMAs
            # never queue behind output DMAs on the sync HWDGE ring; stripes
            # are chunk-aligned so the first matmuls only wait on stripe 0.
            stripes = [(0, 1792), (1792, 3584), (3584, 7168), (7168, 10752), (10752, RHALF)]
            kv_sbs = []
            for b in range(B):
                kv_sb = rhs_pool.tile([128, RHALF], bf16, tag="term", name=f"kv{b}")
                kv_sbs.append(kv_sb)
                for s0, s1 in stripes:
                    nc.scalar.dma_start(kv_sb[:, s0:s1], kv_d[b][:, s0:s1])

            ev = 0
            for b in range(B):
                kv_sb = kv_sbs[b]
                for mg in range(MG):
                    wcol = wt_sb[:, b * MG + mg : b * MG + mg + 1]
                    ot = out_pool.tile([128, REG], bf16, tag="ot")
                    for h in range(2):
                        si = (b * MG + mg) * 2 + h
                        sel_ap = sel_sb[:, si * 128 : (si + 1) * 128]
                        for c in range(NCH):
                            ps = psum_pool.tile([128, CH], f32, tag="ps")
                            for m0 in range(0, CH, MM):
                                mw = min(MM, CH - m0)
                                col = c * CH + m0
                                nc.tensor.matmul(
                                    ps[:, m0 : m0 + mw],
                                    sel_ap,
                                    kv_sb[:, col : col + mw],
                                )
                            dst = ot[:, h * RHALF + c * CH : h * RHALF + (c + 1) * CH]
                            # alternate DVE/ACT 1:1 (both ~110 G elem/s on
                            # f32 PSUM reads)
                            if ev % 2 == 0:
                                nc.vector.tensor_scalar_mul(dst, ps[:], wcol)
                            else:
                                nc.scalar.activation(
                                    dst,
                                    ps[:],
                                    mybir.ActivationFunctionType.Copy,
                                    scale=wcol,
                                )
                            ev += 1
                            # per-chunk output DMA keeps the sync HWDGE ring
                            # always fed (a ready transfer behind the current
                            # one), avoiding inter-DMA bubbles
                            col0 = h * RHALF + c * CH
                            nc.sync.dma_start(
                                out_d[b, mg * 128 : (mg + 1) * 128, col0 : col0 + CH],
                                ot[:, col0 : col0 + CH],
                            )

    nc.compile()
    return nc


def _get_nc():
    global _COMPILED
    if _COMPILED is None:
        _COMPILED = _build()
    return _COMPILED


def _prep_core(kv_c: np.ndarray, idx_c: np.ndarray, w_c: np.ndarray) -> dict:
    """kv_c (B, 64, 196, 128) f32, idx_c (B, 64, 4) int, w_c (B, 64, 4) f32."""
    # rhs layout [B, 128, RHALF]: partition h*64 + r = half h of region r (flat)
    kvb = (
        kv_c.reshape(B, P2, 2, RHALF).transpose(0, 2, 1, 3).reshape(B, 128, RHALF)
    ).astype(BF16)

    idx_f = idx_c.reshape(B, G).astype(np.int64)
    w_f = w_c.reshape(B, G).astype(np.float32)

    sel = np.zeros((128, B, MG, 2, 128), dtype=BF16)
    k = np.arange(128)[:, None]
    for b in range(B):
        for mg in range(MG):
            im = idx_f[b, mg * 128 : (mg + 1) * 128][None, :]
            sel[:, b, mg, 0] = (k == im).astype(BF16)
            sel[:, b, mg, 1] = (k == im + 64).astype(BF16)
    sel = sel.reshape(128, B * MG * 2 * 128)

    wt = np.zeros((128, B * MG), dtype=np.float32)
    for b in range(B):
        for mg in range(MG):
            wt[:, b * MG + mg] = w_f[b, mg * 128 : (mg + 1) * 128]

    return {"kvb": kvb, "sel": sel, "wt": wt}


def kernel(r_idx: np.ndarray, r_weight: np.ndarray, kv: np.ndarray) -> np.ndarray:
    global LAST_RESULTS
    nc = _get_nc()
    kv = np.asarray(kv, dtype=np.float32)
    r_idx = np.asarray(r_idx)
    r_weight = np.asarray(r_weight, dtype=np.float32)

    in_maps = [
        _prep_core(
            kv[c * B : (c + 1) * B],
            r_idx[c * B : (c + 1) * B],
            r_weight[c * B : (c + 1) * B],
        )
        for c in range(N_CORES)
    ]

    res = run_bass_kernel_spmd(nc, in_maps, core_ids=list(range(N_CORES)), **RUN_KWARGS)
    LAST_RESULTS = res

    out = np.empty((N, P2, TOPK, W2, C_KV), dtype=np.float32)
    for c in range(N_CORES):
        o = res.results[c]["out"]  # (B, G, REG) bf16
        out[c * B : (c + 1) * B] = o.astype(np.float32).reshape(B, P2, TOPK, W2, C_KV)
    return out
